# revision 3
# baseline (speedup 1.0000x reference)
"""Trainium2 Bass kernel for nn_Decoder (gnn_message_passing), v2.

Computation (per graph b):
  p1 = node_fts @ W1 + b1                       (N, H)
  p2 = node_fts @ W2 + b2                       (N, H)
  p3 = edge_fts @ W3 + b3                       (N, N, H)
  p_e = p2[:, None, :] + p3                     (j, i, H) view
  p_m[i, j, h] = max(p1[i, h], p_e[j, i, h])
  preds = p_m @ W4 + b4                         (N, N)
  preds = where(adj > .5, preds, min(-1, min(preds) - 1))
  out = log_sinkhorn(preds, 10 steps, temp .1)

Sharding: 8 cores = 4 graphs x 2 column-halves (core even: j 0:128,
odd: j 128:256 of its graph; CORE_MAP below). Within a core, columns
are processed transposed (h on partitions).

Main loop, 128 columns per core as 64 "residues" (column pairs
j = 64*pass + 32*u + c, u in {0,1}):
  - edge_fts shipped fp8e4 pre-packed for DoubleRow: ONE k=256 matmul per
    residue (rhs free = [s=2, (u,i)=512]); 4 consecutive mms share the
    16*W3 lhsT per DMA batch.
  - pm = max(p3 + p2[j], p1) evacuates PSUM via two balanced paths:
    DVE scalar_tensor_tensor directly (23 residues), or ACT identity+bias
    copy to bf16 SBUF then one paired DVE tensor_tensor max at 2x. Only
    DVE/ACT can read PSUM (Pool has no PSUM port; walrus also rejects its
    tensor ops, and every matmul re-emits LDWEIGHTS - no dedupe).
  - ONE 128-wide window matmul per residue: W4/16 sits at strip columns
    (32+par, 64+par); slice [off, off+128) lands it on pacc rows c and
    c+32, each collecting [preds_u0 | preds_u1] for the residue.
  - preds^T ships to the pair core as ONE fp8 AllGather (a cc op costs
    ~13us + bandwidth shared by all 4 pair-groups, so one op beats two);
    the own-half preds min rides in payload row 64, and rearranged-AP
    DMAs unscramble (pass, u, c) into the ptj tiles.

Sinkhorn is computed in factored exp space: P = diag(v) P0 diag(u)
with P0 = exp(X) fixed, so each of the 10 steps is two matvec+recip
rounds on PE/DVE (u = 1/(P0^T v), v = 1/(P0 u)) instead of full-matrix
log-softmax passes. Output y = X + ln v[j] + ln u[i]. This is exact
algebra, not an approximation; bf16 storage of P0/u/v introduces
~0.3 absolute error vs the 1e6-scale output (≈3e-7 relative).
"""

import os
import sys

for _p in ("/opt/trn_rl_repo", "/root/.axon_site/_ro/trn_rl_repo"):
    if os.path.isdir(_p) and _p not in sys.path:
        sys.path.insert(0, _p)

import ml_dtypes
import numpy as np

import concourse.bacc as bacc
import concourse.mybir as mybir
import concourse.tile as tile
from concourse.bass_utils import run_bass_kernel_spmd

# Pin exp/ln/identity to the one table set that holds all three (see v1).
_ORIG_GAT = bacc.get_activation_tables


def _pinned_tables(arch):
    af = mybir.ActivationFunctionType
    pin = {af.Exp, af.Ln, af.Identity, af.Copy}
    out = {}
    for name, funcs in _ORIG_GAT(arch).items():
        if name == "natural_log_exp_and_others":
            out[name] = funcs
        else:
            out[name] = funcs - pin
    return out


bacc.get_activation_tables = _pinned_tables

F32 = mybir.dt.float32
BF16 = mybir.dt.bfloat16
FP8 = mybir.dt.float8e4
AF = mybir.ActivationFunctionType
ALU = mybir.AluOpType
AX = mybir.AxisListType
DRM = mybir.MatmulPerfMode.DoubleRow

B, N, H = 4, 256, 128
ND, ED = 3 * H, 2 * H
JH = N // 2          # columns per core
NEG = -1.0e6
TINV = 10.0
STEPS = 10
SCALE = 16.0         # fp8 scaling of the W3 path; w4 carries 1/SCALE
BF = ml_dtypes.bfloat16
F8 = ml_dtypes.float8_e4m3

# column processing order: j_local = 64*pass + 32*u + c, c-major per pass
ORDER = [64 * p + 32 * u + c for p in (0, 1) for c in range(32) for u in (0, 1)]


def build_nc():
    nc = bacc.Bacc("TRN2", target_bir_lowering=False, debug=True)

    # bf16 constant pack, one DMA: w1(3x128) w2(3x128) nft(3x256) nfh(3x128)
    # w4 strips(2x164) ident(128)  -> 2376 columns
    eft = nc.declare_dram_parameter("eft", [128, 64, 2, 2, N], FP8, isOutput=False)
    cpb = nc.declare_dram_parameter("cpb", [128, 2376], BF16, isOutput=False)
    cpf = nc.declare_dram_parameter("cpf", [128, 3], F32, isOutput=False)
    w3 = nc.declare_dram_parameter("w3", [128, 2, H], FP8, isOutput=False)
    kmq = nc.declare_dram_parameter("kmq", [128, 4 * N], BF16, isOutput=False)
    dgd = nc.declare_dram_parameter("dgd", [128, 2 * N], F32, isOutput=False)
    y = nc.declare_dram_parameter("y", [N, N], F32, isOutput=True)

    with tile.TileContext(nc) as tc:
        with (
            tc.tile_pool(name="const", bufs=1) as cp,
            tc.tile_pool(name="edge", bufs=3) as ep,
            tc.tile_pool(name="pmp", bufs=8) as pmp,
            tc.tile_pool(name="work", bufs=3) as wp,
            tc.tile_pool(name="stat", bufs=2) as st,
            tc.tile_pool(name="psum", bufs=6, space="PSUM") as pp,
            tc.tile_pool(name="acc", bufs=1, space="PSUM") as ap_,
            tc.tile_pool(name="dram", bufs=1, space="DRAM") as dp,
        ):
            # ---- prefetch edge batch 0 first ----
            NB = 16            # dma batches
            RPB = 4            # residues per dma batch (8 columns)
            et0 = ep.tile([128, RPB, 2, 2, N], FP8, tag="et", name="et0")
            nc.sync.dma_start(out=et0[:], in_=eft[:, 0:RPB])

            # ---- constants: two packed DMAs ----
            w3s = cp.tile([128, 2, H], FP8, tag="w3s", name="w3s")
            nc.sync.dma_start(out=w3s[:], in_=w3[:])
            cbs = cp.tile([128, 2376], BF16, tag="cbs", name="cbs")
            nc.sync.dma_start(out=cbs[:], in_=cpb[:])
            cfs = cp.tile([128, 3], F32, tag="cfs", name="cfs")
            nc.sync.dma_start(out=cfs[:], in_=cpf[:])
            w1s = [cbs[:, c * 128:(c + 1) * 128] for c in range(3)]
            w2s = [cbs[:, 384 + c * 128:384 + (c + 1) * 128] for c in range(3)]
            nfts = [cbs[:, 768 + c * N:768 + (c + 1) * N] for c in range(3)]
            nfhs = [cbs[:, 1536 + c * 128:1536 + (c + 1) * 128] for c in range(3)]
            w4t = [cbs[:, 1920 + p * 164:1920 + (p + 1) * 164] for p in range(2)]
            idb = cbs[:, 2248:2376]
            b1s = cfs[:, 0:1]
            b2s = cfs[:, 1:2]
            b4s = cfs[:, 2:3]
            on1 = cp.tile([1, 128], BF16, tag="on1", name="on1")
            nc.vector.memset(on1[:], 1.0)

            # ---- p1T (H, N), p2T (H, JH), scaled by 16 ----
            p1ps = pp.tile([H, N], F32, tag="p3", name="p1ps")
            for c in range(3):
                nc.tensor.matmul(out=p1ps[:], lhsT=w1s[c], rhs=nfts[c],
                                 start=(c == 0), stop=(c == 2))
            p1s = cp.tile([H, N], F32, tag="p1s", name="p1s")
            nc.scalar.activation(out=p1s[:], in_=p1ps[:], func=AF.Identity,
                                 bias=b1s[:], scale=1.0)
            p2ps = pp.tile([H, JH], F32, tag="p3", name="p2ps")
            for c in range(3):
                nc.tensor.matmul(out=p2ps[:], lhsT=w2s[c], rhs=nfhs[c],
                                 start=(c == 0), stop=(c == 2))
            p2s = cp.tile([H, JH], F32, tag="p2s", name="p2s")
            nc.scalar.activation(out=p2s[:], in_=p2ps[:], func=AF.Identity,
                                 bias=b2s[:], scale=1.0)

            # ---- main loop ----
            # pacc tiles own a full 2KB bank row: matmul start=True marks the
            # whole row pending-zero, so a narrower tile would stomp a
            # bank-sharing neighbor.
            pacc = [ap_.tile([128, 512], F32, tag=f"pacc{p}", name=f"pacc{p}",
                             bufs=1) for p in range(2)]
            p1b2 = cp.tile([H, 2 * N], BF16, tag="p1b2", name="p1b2")
            nc.scalar.copy(out=p1b2[:, 0:N], in_=p1s[:])
            nc.scalar.copy(out=p1b2[:, N:2 * N], in_=p1s[:])
            bin_ = dp.tile([65, 2 * N], FP8, tag="bin", name="bin")
            bout = dp.tile([2, 65, 2 * N], FP8, tag="bout", name="bout")
            ptj = [cp.tile([128, N], FP8, tag=f"ptj{t}", name=f"ptj{t}")
                   for t in range(2)]
            psbs = []
            rms = []

            et = et0
            for bt in range(NB):
                if bt + 1 < NB:
                    etn = ep.tile([128, RPB, 2, 2, N], FP8, tag="et", name=f"et{bt+1}")
                    nc.sync.dma_start(out=etn[:], in_=eft[:, (bt + 1) * RPB:(bt + 2) * RPB])
                else:
                    etn = None
                pas = bt // 8
                # this dma batch covers residues c0..c0+3, both u columns each
                c0 = (bt % 8) * 4
                # one k=256 DoubleRow matmul per residue covers both columns
                # (rhs free = [s=2, (u,i)=512]); 4 consecutive mms share w3s
                p3t = [pp.tile([128, 2 * N], F32, tag="p3", name=f"p3_{bt}_{i}")
                       for i in range(4)]
                for i in range(4):
                    nc.tensor.matmul(
                        out=p3t[i][:], lhsT=w3s[:], rhs=et[:, i],
                        start=True, stop=True,
                        perf_mode=DRM, skip_group_check=True)
                # pm = max(p3 + p2[j], p1). Only DVE/ACT can read PSUM (Pool
                # has no PSUM port). DVE-path: direct stt per column (f32
                # PSUM in, ~480ns). ACT-path: identity+bias copy to SBUF bf16
                # per column (~480ns on ACT), then one paired bf16
                # tensor_tensor max on DVE at 2x (~330ns per residue).
                pms = []
                for i in range(4):
                    cc = c0 + i
                    on_dve = (cc % 8 in (0, 3, 6)) and cc != 59
                    pm = pmp.tile([128, 2 * N], BF16, tag="pm",
                                  name=f"pm_{bt}_{i}")
                    if on_dve:
                        for u in range(2):
                            jl = 64 * pas + 32 * u + cc
                            nc.vector.scalar_tensor_tensor(
                                out=pm[:, u * N:(u + 1) * N],
                                in0=p3t[i][:, u * N:(u + 1) * N],
                                scalar=p2s[:, jl:jl + 1], in1=p1s[:],
                                op0=ALU.add, op1=ALU.max)
                    else:
                        pe_sb = pmp.tile([128, 2 * N], BF16, tag="pe",
                                         name=f"pe_{bt}_{i}")
                        for u in range(2):
                            jl = 64 * pas + 32 * u + cc
                            nc.scalar.activation(
                                out=pe_sb[:, u * N:(u + 1) * N],
                                in_=p3t[i][:, u * N:(u + 1) * N],
                                func=AF.Identity, bias=p2s[:, jl:jl + 1],
                                scale=1.0)
                        nc.vector.tensor_tensor(
                            out=pm[:], in0=pe_sb[:], in1=p1b2[:],
                            op=ALU.max)
                    pms.append(pm)
                # one 128-wide window matmul per residue: w4 sits at strip
                # cols (32+par, 64+par); slice [off, off+128) puts it at
                # locals (c, c+32), so pacc row c and c+32 both collect
                # [preds_{u0} | preds_{u1}] of residue c. Row c of pacc =
                # row c+32; rows 0:32 are read out.
                for i in range(4):
                    cc = c0 + i
                    par = cc % 2
                    off = (32 + par) - cc
                    nc.tensor.matmul(
                        out=pacc[pas][:, :],
                        lhsT=w4t[par][:, off:off + 128],
                        rhs=pms[i][:],
                        start=(bt % 8 == 0 and i == 0),
                        stop=(bt % 8 == 7 and i == 3),
                        skip_group_check=True)
                et = etn
                if bt == 7 or bt == 15:
                    # pass done: stage preds^T chunk into the exchange buffer.
                    # A single AllGather at the end beats two: each cc op has
                    # a ~13-15us floor and they serialize on the cc stream.
                    psb = st.tile([32, 2 * N], FP8, tag=f"psb{pas}", name=f"psb{pas}")
                    nc.scalar.copy(out=psb[:], in_=pacc[pas][0:32, :])
                    psbs.append(psb)
                    nc.gpsimd.dma_start(out=bin_[32 * pas:32 * pas + 32, :], in_=psb[:])
                    rm = st.tile([32, 1], F32, tag=f"rm{pas}", name=f"rm{pas}")
                    nc.vector.tensor_reduce(out=rm[:], in_=psb[:], axis=AX.X,
                                            op=ALU.min)
                    rms.append(rm)
                if bt == 8:
                    # masks arrive during pass 1 (two packed DMAs)
                    kqs = cp.tile([128, 4 * N], BF16, tag="kqs", name="kqs")
                    nc.sync.dma_start(out=kqs[:], in_=kmq[:])
                    dgs2 = cp.tile([128, 2 * N], F32, tag="dgs2", name="dgs2")
                    nc.sync.dma_start(out=dgs2[:], in_=dgd[:])
                    kms = [kqs[:, t * N:(t + 1) * N] for t in range(2)]
                    qms = [kqs[:, 2 * N + t * N:2 * N + (t + 1) * N] for t in range(2)]
                    dgs = [dgs2[:, t * N:(t + 1) * N] for t in range(2)]

            # fold own-half mins to one fp8 scalar in bin row 64
            rc = st.tile([32, 1], BF16, tag="rc", name="rc")
            nc.vector.tensor_tensor(out=rc[:], in0=rms[0][:], in1=rms[1][:],
                                    op=ALU.min)
            rt = pp.tile([1, 32], BF16, tag="p3", name="rt")
            nc.tensor.transpose(rt[:], rc[:], idb[0:32, 0:32])
            lm1 = st.tile([1, 1], F32, tag="lm1", name="lm1")
            nc.vector.tensor_reduce(out=lm1[:], in_=rt[:], axis=AX.X, op=ALU.min)
            # replicate to a full row (the collective ships whole rows)
            lmr = st.tile([1, 2 * N], FP8, tag="lmr", name="lmr")
            nc.vector.memset(lmr[:], 0.0)
            nc.vector.tensor_scalar(out=lmr[:], in0=lmr[:], scalar1=lm1[:],
                                    scalar2=None, op0=ALU.add)
            nc.gpsimd.dma_start(out=bin_[64:65, :], in_=lmr[:])

            nc.gpsimd.collective_compute(
                "AllGather", ALU.bypass,
                replica_groups=[[0, 2], [1, 3], [4, 6], [5, 7]],
                ins=[bin_.opt()], outs=[bout.opt()])
            # pmin scalars first: they gate the fill -> X chain
            pmin2 = st.tile([1, 2], FP8, tag="pmin2", name="pmin2")
            nc.sync.dma_start(out=pmin2[:],
                              in_=bout[:, 64:65, 0:1].rearrange("r a b -> a (r b)"))
            # unscramble: ptj[r] row 64*pas+32*u+c <- bout[r][32*pas+c, u-block]
            for r in range(2):
                for pas in range(2):
                    src_ap = bout[r][32 * pas:32 * pas + 32, :].rearrange(
                        "c (u i) -> u c i", u=2)
                    nc.sync.dma_start(out=ptj[r][64 * pas:64 * pas + 64, :],
                                      in_=src_ap)

            # ---- pmin -> fill (fcol = TINV * min(-1, pmin + b4 - 1)) ----
            pm1 = st.tile([1, 1], F32, tag="pm1", name="pm1")
            nc.vector.tensor_reduce(out=pm1[:], in_=pmin2[:], axis=AX.X, op=ALU.min)
            f1 = st.tile([1, 1], F32, tag="f1", name="f1")
            nc.vector.tensor_scalar(out=f1[:], in0=pm1[:], scalar1=b4s[0:1],
                                    scalar2=-1.0, op0=ALU.add, op1=ALU.add)
            f2 = st.tile([1, 1], BF16, tag="f2", name="f2")
            nc.vector.tensor_scalar(out=f2[:], in0=f1[:], scalar1=-1.0,
                                    scalar2=TINV, op0=ALU.min, op1=ALU.mult)
            fps = pp.tile([128, 1], F32, tag="p3", name="fps")
            nc.tensor.matmul(out=fps[:], lhsT=on1[:], rhs=f2[:], start=True, stop=True)
            fcol = st.tile([128, 1], F32, tag="fcol", name="fcol")
            nc.scalar.copy(out=fcol[:], in_=fps[:])

            # ---- X = km*(predsT+b4) + qm*fill10 + dg   (bf16) ----
            xs = []
            for t in range(2):
                t2 = wp.tile([128, N], BF16, tag="t2", name=f"t2_{t}")
                nc.vector.scalar_tensor_tensor(out=t2[:], in0=ptj[t][:],
                                               scalar=b4s[:], in1=kms[t][:],
                                               op0=ALU.add, op1=ALU.mult)
                t3 = wp.tile([128, N], F32, tag="t3", name=f"t3_{t}")
                nc.vector.scalar_tensor_tensor(out=t3[:], in0=qms[t][:],
                                               scalar=fcol[:], in1=dgs[t][:],
                                               op0=ALU.mult, op1=ALU.add)
                xx = cp.tile([128, N], F32, tag=f"x{t}", name=f"x{t}")
                nc.vector.tensor_tensor(out=xx[:], in0=t2[:], in1=t3[:], op=ALU.add)
                xs.append(xx)

            # ---- P0 = exp(X), P0^T quadrants ----
            p0s = []
            for t in range(2):
                p0 = cp.tile([128, N], BF16, tag=f"p0_{t}", name=f"p0_{t}")
                nc.scalar.activation(out=p0[:], in_=xs[t][:], func=AF.Exp)
                p0s.append(p0)
            p0ts = [cp.tile([128, N], BF16, tag=f"p0t_{b}", name=f"p0t_{b}")
                    for b in range(2)]

            def do_transposes():
                # runs on PE between the first u-update and the first
                # v-update, hiding the transpose latency in the chain
                for b in range(2):
                    for a in range(2):
                        tp = pp.tile([128, 128], BF16, tag="p3", name=f"tp{b}{a}")
                        nc.tensor.transpose(tp[:], p0s[a][:, b * 128:(b + 1) * 128], idb)
                        if a == 0:
                            nc.vector.tensor_copy(out=p0ts[b][:, a * 128:(a + 1) * 128], in_=tp[:])
                        else:
                            nc.scalar.copy(out=p0ts[b][:, a * 128:(a + 1) * 128], in_=tp[:])

            # ---- sinkhorn: u = 1/(P0^T v), v = 1/(P0 u), 10 steps ----
            vr = st.tile([128, 2], BF16, tag="vr", name="vr_init")
            nc.vector.memset(vr[:], 1.0)
            up = ur = vp = None
            with nc.allow_low_precision(reason="sinkhorn vectors tolerate bf16"):
                for step in range(STEPS):
                    up = pp.tile([128, 2], F32, tag="p3", name=f"up{step}")
                    for b_ in range(2):
                        for a in range(2):
                            nc.tensor.matmul(
                                out=up[:, b_:b_ + 1],
                                lhsT=p0s[a][:, b_ * 128:(b_ + 1) * 128],
                                rhs=vr[:, a:a + 1],
                                start=(a == 0), stop=(a == 1),
                                skip_group_check=True)
                    if step == 0:
                        do_transposes()
                    ur = st.tile([128, 2], BF16, tag="ur", name=f"ur{step}")
                    nc.vector.reciprocal(out=ur[:], in_=up[:])
                    vp = pp.tile([128, 2], F32, tag="p3", name=f"vp{step}")
                    for a in range(2):
                        for b_ in range(2):
                            nc.tensor.matmul(
                                out=vp[:, a:a + 1],
                                lhsT=p0ts[b_][:, a * 128:(a + 1) * 128],
                                rhs=ur[:, b_:b_ + 1],
                                start=(b_ == 0), stop=(b_ == 1),
                                skip_group_check=True)
                    vr = st.tile([128, 2], BF16, tag="vr", name=f"vr{step}")
                    nc.vector.reciprocal(out=vr[:], in_=vp[:])

            # ---- y = X + ln v[j] + ln u[i] ----
            lnu = st.tile([128, 2], BF16, tag="lnu", name="lnu")
            nc.scalar.activation(out=lnu[:], in_=ur[:], func=AF.Ln)
            lnv = st.tile([128, 2], F32, tag="lnv", name="lnv")
            nc.scalar.activation(out=lnv[:], in_=vr[:], func=AF.Ln)
            lnup = pp.tile([1, N], BF16, tag="p3", name="lnup")
            for b_ in range(2):
                nc.tensor.transpose(lnup[:, b_ * 128:(b_ + 1) * 128],
                                    lnu[:, b_:b_ + 1], idb)
            lnur = st.tile([1, N], BF16, tag="lnur", name="lnur")
            nc.scalar.copy(out=lnur[:], in_=lnup[:])
            ypad = pp.tile([128, N], F32, tag="p3", name="ypad")
            for b_ in range(2):
                nc.tensor.matmul(out=ypad[:, b_ * 128:(b_ + 1) * 128],
                                 lhsT=on1[:], rhs=lnur[:, b_ * 128:(b_ + 1) * 128],
                                 start=(b_ == 0), stop=(b_ == 1),
                                 skip_group_check=True)
            ysb = wp.tile([128, 2 * N], F32, tag="ysb", name="ysb")
            for a in range(2):
                nc.vector.scalar_tensor_tensor(
                    out=ysb[:, a * N:(a + 1) * N], in0=xs[a][:],
                    scalar=lnv[:, a:a + 1],
                    in1=ypad[:], op0=ALU.add, op1=ALU.add)
            nc.sync.dma_start(
                out=y[:].rearrange("(a j) i -> j a i", a=2), in_=ysb[:])

    nc.finalize()
    return nc


_NC = None


def _get_nc():
    global _NC
    if _NC is None:
        _NC = build_nc()
    return _NC


CORE_MAP = {0: (0, 0), 2: (0, 1), 1: (1, 0), 3: (1, 1),
            4: (2, 0), 6: (2, 1), 5: (3, 0), 7: (3, 1)}


def _prep_core(c, node_fts, edge_fts, adj_mat, W1, b1, W2, b2, W3, b3, W4, b4):
    b, half = CORE_MAP[c]
    j0 = half * JH
    # edge_fts -> fp8 DoubleRow layout [p, resid, s, u, i], k = p + 128*s,
    # resid = (pass, c); the (u, i) free dims merge into the 512-wide rhs
    ef = edge_fts[b, j0 + np.asarray(ORDER)]        # (t, i, k), t=(pass,c,u)
    eftv = np.ascontiguousarray(
        ef.reshape(64, 2, N, 2, 128)                # (resid, u, i, s, p)
        .transpose(4, 0, 3, 1, 2)).astype(F8)       # (p, resid, s, u, i)
    nftT = (node_fts[b].T * SCALE).astype(np.float32)   # (ND, N), scaled
    eye = np.eye(N, dtype=bool)
    adjT = adj_mat[b].T                             # (j, i)
    kmv = np.where((adjT > 0.5) & ~eye, TINV, 0.0)
    qmv = np.where((adjT <= 0.5) & ~eye, 1.0, 0.0)
    dgv = np.where(eye, NEG, 0.0)
    w4sv = np.zeros((2, H, 164), np.float32)
    for par in range(2):
        w4sv[par, :, 32 + par] = W4[:, 0] / SCALE
        w4sv[par, :, 64 + par] = W4[:, 0] / SCALE
    w3v = np.ascontiguousarray(
        (W3 * SCALE).reshape(2, 128, H).transpose(1, 0, 2)).astype(F8)
    # bf16 constant pack: w1(3x128) w2(3x128) nft(3x256) nfh(3x128)
    # w4 strips(2x164) ident(128)
    w1r = W1.reshape(3, 128, H)
    w2r = W2.reshape(3, 128, H)
    nftr = nftT.reshape(3, 128, N)
    nfhr = np.ascontiguousarray(nftT[:, j0:j0 + JH]).reshape(3, 128, JH)
    cpbv = np.concatenate(
        [w1r[0], w1r[1], w1r[2], w2r[0], w2r[1], w2r[2],
         nftr[0], nftr[1], nftr[2], nfhr[0], nfhr[1], nfhr[2],
         w4sv[0], w4sv[1], np.eye(128, dtype=np.float32)], axis=1)
    cpfv = np.stack([b1 * SCALE, (b2 + b3) * SCALE,
                     np.full(128, float(b4[0]))], axis=1)
    kmr = kmv.reshape(2, 128, N)
    qmr = qmv.reshape(2, 128, N)
    dgr = dgv.reshape(2, 128, N)
    return {
        "eft": eftv,
        "cpb": cpbv.astype(BF),
        "cpf": cpfv.astype(np.float32),
        "w3": w3v,
        "kmq": np.concatenate([kmr[0], kmr[1], qmr[0], qmr[1]], axis=1).astype(BF),
        "dgd": np.concatenate([dgr[0], dgr[1]], axis=1).astype(np.float32),
    }


def kernel(node_fts, edge_fts, adj_mat, W1, b1, W2, b2, W3, b3, W4, b4,
           _trace=False):
    args = [np.asarray(a) for a in
            (node_fts, edge_fts, adj_mat, W1, b1, W2, b2, W3, b3, W4, b4)]
    nc = _get_nc()
    in_maps = [_prep_core(c, *args) for c in range(8)]
    res = run_bass_kernel_spmd(nc, in_maps, core_ids=list(range(8)),
                               trace=_trace)
    out = np.stack([res.results[g]["y"].T for g in (0, 1, 4, 5)])
    if _trace:
        kernel.last_exec_time_ns = res.exec_time_ns
    return out.astype(np.float32)
